# revision 39
# baseline (speedup 1.0000x reference)
"""Trainium2 Bass kernel for nn_CAMLocalBiHead (CAM scoring + topk mask + conv + BCE).

Strategy: pure data parallel over batch (8 samples per NeuronCore x 8 cores).
Each core computes per-sample BCE partial sums; host sums and divides.

Per-core pipeline:
  - conv weights [512,2304] f32 -> PE transposes -> lhsT layout [C,9,D] bf16
  - x pre-cast to bf16 on host, DMA'd straight into the [C, b, T, 7, 7] layout
  - CAM: argmax via reduce_max+is_ge one-hot, one-hot matmul gathers w_proj row,
    per-sample row einsum on PE (both cams in one M=2 matmul), min-max
    normalize, top-392 threshold by bisection (count = fused is_ge+accum DVE
    pass), mask = (r>=lo)*r, y = max(v, n)
  - conv 3x3 (C=256->D=512): 18 accumulating matmuls per [128, 392] PSUM tile
    with pad-skip tap subranges (boundary taps omitted == zero padding exactly)
  - relu+bias fused in ACT PSUM->SBUF evacuation (bf16)
  - 1x1 score conv: 4 accumulating [128,1]x[128,392] matmuls, deferred one
    group so PE never waits on ACT; nt-major order so BCE chunks overlap conv
  - BCE: sum(softplus(x) - y*x), softplus = relu(x) + ln(1+exp(-|x|)) on ACT
    with fused per-partition accumulation; emitted per column chunk inline
"""

import numpy as np

import concourse.bass as bass
import concourse.mybir as mybir

_BF_NP = None  # set below once mybir is imported
from concourse.tile import TileContext
from concourse.tile_rust import add_dep_helper
from concourse.masks import make_identity
from concourse.bass_utils import run_bass_kernel_spmd

FP = mybir.dt.float32
BF = mybir.dt.bfloat16
AF = mybir.ActivationFunctionType
OP = mybir.AluOpType
AX = mybir.AxisListType
_BF_NP = mybir.dt.np(mybir.dt.bfloat16)

N_CORES = 8
B_FULL = 64
C = 256
T = 32
H = 7
W = 7
HW = H * W
D = 512
KV = 97
KN = 300
NTOK = 392
B8 = B_FULL // N_CORES
KT = C // 128          # 2 contraction tiles
MT = D // 128          # 4 output-channel tiles
TCH = 8                # t-slices per spatial chunk
BISECT_ITERS = 14


def build_bass(b8=B8, t=T, ntok=NTOK, bisect_iters=BISECT_ITERS, pad_skip=True):
    n = t * HW                    # spatial positions per sample
    ntc = t // TCH                # number of spatial chunks
    chunk = TCH * HW              # 392 positions per chunk

    nc = bass.Bass()
    x8 = nc.declare_dram_parameter("x8", [b8, C, n], BF, isOutput=False)
    pv = nc.declare_dram_parameter("pv", [b8, KV], FP, isOutput=False)
    pn = nc.declare_dram_parameter("pn", [b8, KN], FP, isOutput=False)
    wpv = nc.declare_dram_parameter("wpv", [KV, C], FP, isOutput=False)
    wpn = nc.declare_dram_parameter("wpn", [KN, C], FP, isOutput=False)
    cwT = nc.declare_dram_parameter("cwT", [C, 9 * D], BF, isOutput=False)
    cbt = nc.declare_dram_parameter("cbt", [128, MT], FP, isOutput=False)
    swt = nc.declare_dram_parameter("swt", [128, MT], FP, isOutput=False)
    sb11 = nc.declare_dram_parameter("sb11", [1, 1], FP, isOutput=False)
    out = nc.declare_dram_parameter("out", [b8, 1], FP, isOutput=True)

    # pad-skip ranges per kernel tap: output h in [oh0, oh1), input h = h+ky-1
    def ranges(k):
        o0 = max(0, 1 - k)
        o1 = min(7, 8 - k)
        return o0, o1

    with TileContext(nc) as tc:
        with tc.tile_pool(name="cp", bufs=1) as cp:
            # ---- persistent tiles ----
            idn8 = cp.tile([b8, b8], FP)
            make_identity(nc, idn8[:])

            wT = [cp.tile([128, 9, D], BF, name=f"wT{k}") for k in range(KT)]
            # pad_skip: unpadded layout, boundary taps use restricted ranges
            # (exactly equivalent to zero padding). else: 9x9 zero-padded.
            PW = H if pad_skip else H + 2
            xbf = [cp.tile([128, b8, t, PW, PW], BF, name=f"xbf{k}")
                   for k in range(KT)]
            xoff = 0 if pad_skip else 1
            cb_t = cp.tile([128, MT], FP)
            sw_t = cp.tile([128, MT], FP)
            sT = cp.tile([128, MT], BF)
            sb_t = cp.tile([1, 1], FP)
            pv_t = cp.tile([b8, KV], FP)
            pn_t = cp.tile([b8, KN], FP)
            wpv_t = cp.tile([KV, C], FP)
            wpn_t = cp.tile([128, 3, C], FP)
            ohv = cp.tile([b8, KV], FP)
            ohn = cp.tile([b8, KN], FP)
            ohvT = cp.tile([KV, b8], FP)
            ohnT = cp.tile([128, 3, b8], FP)
            wtop = [cp.tile([b8, C], FP, name=f"wtop{i}") for i in range(2)]
            wtTa = cp.tile([128, KT, b8, 2], BF)
            r2 = 2 * b8
            rows = cp.tile([r2, n], FP)   # v rows first b8 partitions, n rows next
            cam16 = cp.tile([r2, n], FP)
            camn_s = cp.tile([b8, n], FP)
            y_t = cp.tile([b8, n], FP)
            xlog = cp.tile([b8, n], FP)
            # small stats (16 rows = both cams)
            mn16 = cp.tile([r2, 1], FP)
            mx16 = cp.tile([r2, 1], FP)
            rcp16 = cp.tile([r2, 1], FP)
            lo16 = cp.tile([r2, 1], FP)
            hi16 = cp.tile([r2, 1], FP)
            mid16 = cp.tile([r2, 1], FP)
            cnt16 = cp.tile([r2, 1], FP)
            prd16 = cp.tile([r2, 1], mybir.dt.uint32)
            vmv = cp.tile([b8, 1], FP)
            vmn = cp.tile([b8, 1], FP)
            scr_r = cp.tile([b8, chunk], FP)
            scr_l = cp.tile([b8, chunk], FP)
            scr_x = cp.tile([b8, chunk], FP)
            acc4r = cp.tile([b8, ntc], FP)
            acc4l = cp.tile([b8, ntc], FP)
            acc4x = cp.tile([b8, ntc], FP)
            partial = cp.tile([b8, 1], FP)

            # ---- small input DMAs ----
            nc.sync.dma_start(out=cb_t[:], in_=cbt[:])
            nc.sync.dma_start(out=sw_t[:], in_=swt[:])
            nc.sync.dma_start(out=sb_t[:], in_=sb11[:])
            nc.sync.dma_start(out=pv_t[:], in_=pv[:])
            nc.sync.dma_start(out=pn_t[:], in_=pn[:])
            nc.sync.dma_start(out=wpv_t[:], in_=wpv[:])
            kn_cnt = [128, 128, KN - 256]
            for i in range(3):
                nc.sync.dma_start(
                    out=wpn_t[0:kn_cnt[i], i, :], in_=wpn[128 * i:128 * i + kn_cnt[i], :]
                )
            nc.vector.tensor_copy(sT[:], sw_t[:])
            if not pad_skip:
                for kt in range(KT):
                    nc.vector.memset(xbf[kt][:], 0.0)

            # ---- PE warm-up: dummy accumulating matmuls on a zeroed tile
            # warm the HAM clock gate (4/8 -> 8/8) while input DMAs are still
            # in flight; transposes and DMA waits don't count as PE-busy, so
            # without this the whole CAM phase runs at 1.2 GHz. A second
            # burst after CAM prep keeps PE busy across the dependency wait
            # so the MID window never re-throttles.
            wup_cm = tc.tile_pool(name="wup", bufs=1, space="PSUM")
            wup = wup_cm.__enter__()
            wsrc = cp.tile([128, 512], BF)
            nc.vector.memset(wsrc[:], 0.0)
            wps = wup.tile([128, 512], FP)

            def warm_burst(k):
                for i in range(k):
                    nc.tensor.matmul(
                        wps[:], wsrc[:, 0:128], wsrc[:],
                        start=(i == 0), stop=(i == k - 1),
                    )

            warm_burst(24)

            # ---- weights arrive pre-transposed (and bf16) from the host ----
            for kt in range(KT):
                nc.sync.dma_start(
                    out=wT[kt][:].rearrange("p k d -> p (k d)"),
                    in_=cwT[128 * kt:128 * (kt + 1), :],
                )

            # ---- phase pool: CAM prep ----
            with tc.tile_pool(name="pst", bufs=4, space="PSUM") as pst:
                # CAM prep: argmax one-hot -> w_top -> transposed lhsT
                nc.vector.tensor_reduce(vmv[:], pv_t[:], axis=AX.X, op=OP.max)
                nc.vector.tensor_scalar(ohv[:], pv_t[:], vmv[:], None, op0=OP.is_ge)
                nc.vector.tensor_reduce(vmn[:], pn_t[:], axis=AX.X, op=OP.max)
                nc.vector.tensor_scalar(ohn[:], pn_t[:], vmn[:], None, op0=OP.is_ge)

                psv = pst.tile([KV, b8], FP, tag="pst")
                nc.tensor.transpose(out=psv[:], in_=ohv[:], identity=idn8[:])
                nc.vector.tensor_copy(ohvT[:], psv[:])
                for i in range(3):
                    psn = pst.tile([128, b8], FP, tag="pst")
                    nc.tensor.transpose(
                        out=psn[0:kn_cnt[i], :], in_=ohn[:, 128 * i:128 * i + kn_cnt[i]],
                        identity=idn8[:],
                    )
                    nc.vector.tensor_copy(ohnT[0:kn_cnt[i], i, :], psn[0:kn_cnt[i], :])

                warm_burst(14)
                pw = pst.tile([b8, C], FP, tag="pst")
                nc.tensor.matmul(pw[:], ohvT[:], wpv_t[:], start=True, stop=True)
                nc.vector.tensor_copy(wtop[0][:], pw[:])
                pw2 = pst.tile([b8, C], FP, tag="pst")
                for i in range(3):
                    nc.tensor.matmul(
                        pw2[:], ohnT[0:kn_cnt[i], i, :], wpn_t[0:kn_cnt[i], i, :],
                        start=(i == 0), stop=(i == 2),
                    )
                nc.vector.tensor_copy(wtop[1][:], pw2[:])

                for cam in range(2):
                    for kt in range(KT):
                        pt = pst.tile([128, b8], FP, tag="pst")
                        nc.tensor.transpose(
                            out=pt[:], in_=wtop[cam][:, 128 * kt:128 * (kt + 1)],
                            identity=idn8[:],
                        )
                        nc.scalar.copy(out=wtTa[:, kt, :, cam], in_=pt[:])

            # final filler before the CAM row matmuls
            warm_burst(6)
            wup_cm.__exit__(None, None, None)

            # ---- main loop pools ----
            with (
                tc.tile_pool(name="rowp", bufs=2, space="PSUM") as rowp,
                tc.tile_pool(name="convp", bufs=4, space="PSUM") as convp,
                tc.tile_pool(name="scp", bufs=2, space="PSUM") as scp,
                tc.tile_pool(name="h1p", bufs=12) as h1p,
                tc.tile_pool(name="bncp", bufs=4) as bncp,
            ):
                pending = []
                last_dma = None
                last_pe = None
                last_act = None
                last_dve = None

                def bce_chunk(c):
                    # BCE over column chunk c of all samples: emitted inline
                    # so ACT/DVE process it while conv continues (strict FIFO)
                    nonlocal last_act, last_dve
                    sl = slice(chunk * c, chunk * (c + 1))
                    nc.scalar.activation(
                        out=scr_r[:], in_=xlog[:, sl], func=AF.Relu,
                        accum_out=acc4r[:, c:c + 1],
                    )
                    nc.scalar.activation(
                        out=scr_l[:], in_=xlog[:, sl], func=AF.Abs
                    )
                    nc.scalar.activation(
                        out=scr_l[:], in_=scr_l[:], func=AF.Exp, scale=-1.0,
                    )
                    last_act = nc.scalar.activation(
                        out=scr_l[:], in_=scr_l[:], func=AF.Ln,
                        bias=1.0, accum_out=acc4l[:, c:c + 1],
                    )
                    nc.vector.tensor_tensor(
                        scr_x[:], y_t[:, sl], xlog[:, sl], op=OP.mult
                    )
                    last_dve = nc.vector.tensor_reduce(
                        acc4x[:, c:c + 1], scr_x[:], axis=AX.X, op=OP.add
                    )

                def emit_score(grp):
                    nonlocal last_pe, last_act, last_dma
                    gb, gnt, h1s = grp
                    sp_ps = scp.tile([1, chunk], FP, tag="scps")
                    for mt in range(MT):
                        last_pe = nc.tensor.matmul(
                            sp_ps[:], sT[:, mt:mt + 1], h1s[mt][:],
                            start=(mt == 0), stop=(mt == MT - 1),
                        )
                    # compute-engine SBUF writes must start at partition
                    # 0/32/64/96, so evacuate to a partition-0 bounce tile and
                    # DMA-shift into xlog[gb]
                    xb = bncp.tile([1, chunk], FP, tag="xb", name="xb")
                    last_act = nc.scalar.activation(
                        out=xb[:], in_=sp_ps[:], func=AF.Identity,
                        bias=sb_t[0:1, 0:1],
                    )
                    last_dma = nc.sync.dma_start(
                        out=xlog[gb:gb + 1, chunk * gnt:chunk * (gnt + 1)],
                        in_=xb[:],
                    )
                    if gb == b8 - 1:
                        bce_chunk(gnt)

                for b in range(b8):
                    # x arrives pre-cast to bf16 from the host; DMA straight
                    # into the conv layout
                    for kt in range(KT):
                        if pad_skip:
                            last_dma = nc.sync.dma_start(
                                out=xbf[kt][:, b, :, :, :]
                                    .rearrange("p t h w -> p (t h w)"),
                                in_=x8[b, 128 * kt:128 * (kt + 1), :],
                            )
                        else:
                            for ti in range(t):
                                last_dma = nc.sync.dma_start(
                                    out=xbf[kt][:, b, ti, xoff:xoff + H,
                                                xoff:xoff + W],
                                    in_=x8[b, 128 * kt:128 * (kt + 1),
                                           HW * ti:HW * (ti + 1)]
                                        .rearrange("p (h w) -> p h w", w=W),
                                )

                    # CAM row einsum: both cams in one matmul (M=2)
                    rb = bncp.tile([2, n], FP, tag="rb")
                    for nt in range(ntc):
                        rp = rowp.tile([2, chunk], FP, tag="rowps")
                        for kt in range(KT):
                            nc.tensor.matmul(
                                rp[:], wtTa[:, kt, b, :],
                                xbf[kt][:, b, TCH * nt:TCH * (nt + 1),
                                        xoff:xoff + H, xoff:xoff + W],
                                start=(kt == 0), stop=(kt == KT - 1),
                            )
                        nc.scalar.copy(
                            out=rb[0:2, chunk * nt:chunk * (nt + 1)], in_=rp[:]
                        )
                    nc.sync.dma_start(out=rows[b:b + 1, :], in_=rb[0:1, :])
                    nc.sync.dma_start(
                        out=rows[b8 + b:b8 + b + 1, :], in_=rb[1:2, :]
                    )

                # ---- CAM stats: normalize + bisection threshold + mask ----
                nc.vector.tensor_reduce(mn16[:], rows[:], axis=AX.X, op=OP.min)
                nc.vector.tensor_reduce(mx16[:], rows[:], axis=AX.X, op=OP.max)
                nc.vector.tensor_tensor(rcp16[:], mx16[:], mn16[:], op=OP.subtract)
                nc.vector.reciprocal(rcp16[:], rcp16[:])
                nc.vector.tensor_scalar(
                    rows[:], rows[:], mn16[:], rcp16[:],
                    op0=OP.subtract, op1=OP.mult,
                )
                nc.vector.memset(lo16[:], 0.0)
                nc.vector.memset(hi16[:], 1.0)
                for _ in range(bisect_iters):
                    nc.vector.tensor_tensor(mid16[:], lo16[:], hi16[:], op=OP.add)
                    nc.vector.tensor_scalar_mul(mid16[:], mid16[:], 0.5)
                    nc.vector.tensor_scalar(
                        cam16[:], rows[:], mid16[:], None, op0=OP.is_ge,
                        op1=OP.add, accum_out=cnt16[:],
                    )
                    nc.vector.tensor_scalar(
                        prd16[:], cnt16[:], float(ntok), None, op0=OP.is_ge
                    )
                    nc.vector.copy_predicated(lo16[:], prd16[:], mid16[:])
                    nc.vector.tensor_scalar(
                        prd16[:], cnt16[:], float(ntok), None, op0=OP.is_lt
                    )
                    nc.vector.copy_predicated(hi16[:], prd16[:], mid16[:])
                # cam = (r >= lo) * r for both cams at once
                nc.vector.scalar_tensor_tensor(
                    out=cam16[:], in0=rows[:], scalar=lo16[:], in1=rows[:],
                    op0=OP.is_ge, op1=OP.mult,
                )
                # shift n-cam rows to partitions 0-7, then y = max(v, n)
                shift_dma = nc.sync.dma_start(out=camn_s[:], in_=cam16[b8:r2, :])
                nc.vector.tensor_tensor(y_t[:], cam16[0:b8, :], camn_s[:], op=OP.max)

                # ---- conv 3x3 + deferred 1x1 score, all samples ----
                # (emitted after the CAM chain so the bisection DVE work
                # overlaps conv matmuls instead of trailing the kernel)
                taps = [(1, 1)] + [(ky, kx) for ky in range(3)
                                   for kx in range(3) if (ky, kx) != (1, 1)]
                for nt in range(ntc):
                    for b in range(b8):
                        if pending:
                            emit_score(pending.pop())
                        h1s = []
                        for mt in range(MT):
                            cps = convp.tile([128, chunk], FP, tag="cvps")
                            cpv = cps.rearrange("p (t h w) -> p t h w", h=H, w=W)
                            ntaps = 9 * KT
                            i = 0
                            for ky, kx in taps:
                                if pad_skip:
                                    oh0, oh1 = ranges(ky)
                                    ow0, ow1 = ranges(kx)
                                else:
                                    oh0, oh1, ow0, ow1 = 0, H, 0, W
                                for kt in range(KT):
                                    nc.tensor.matmul(
                                        cpv[:, :, oh0:oh1, ow0:ow1],
                                        wT[kt][:, 3 * ky + kx,
                                               128 * mt:128 * (mt + 1)],
                                        xbf[kt][:, b, TCH * nt:TCH * (nt + 1),
                                                xoff + oh0 + ky - 1:xoff + oh1 + ky - 1,
                                                xoff + ow0 + kx - 1:xoff + ow1 + kx - 1],
                                        start=(i == 0), stop=(i == ntaps - 1),
                                    )
                                    i += 1
                            h1t = h1p.tile([128, chunk], BF, tag="h1")
                            nc.scalar.activation(
                                out=h1t[:], in_=cps[:], func=AF.Relu,
                                bias=cb_t[:, mt:mt + 1], scale=1.0,
                            )
                            h1s.append(h1t)
                        pending.append((b, nt, h1s))
                if pending:
                    emit_score(pending.pop())

                # ---- final reduction of per-chunk BCE accumulators ----
                nc.vector.tensor_tensor(acc4r[:], acc4r[:], acc4l[:], op=OP.add)
                nc.vector.tensor_tensor(acc4r[:], acc4r[:], acc4x[:],
                                        op=OP.subtract)
                last_dve = nc.vector.tensor_reduce(
                    partial[:], acc4r[:], axis=AX.X, op=OP.add
                )
                out_dma = nc.sync.dma_start(out=out[:], in_=partial[:])

                tail = [last_dma, shift_dma, last_pe, last_act,
                        last_dve, out_dma]

                # funnel every engine's final tick through single-wait SP nops
                # so the TileContext tail drain needs <=2 sem waits (walrus
                # rejects instructions with more)
                prev = None
                for dep in tail:
                    if dep is None:
                        continue
                    nop = nc.sync.nop()
                    add_dep_helper(nop.ins, dep.ins, True, "tail funnel")
                    if prev is not None:
                        add_dep_helper(nop.ins, prev.ins, False, "tail chain")
                    prev = nop
    return nc


def _split_excess_waits(nc):
    """Walrus codegen rejects instructions with more sem waits than their
    ISA ctrl struct can hold (1 for Matmult via the LDWEIGHTS struct, ~2
    elsewhere). Hoist excess waits onto same-engine NOPs inserted right
    before the overloaded instruction (engine blocks on the NOP's waits
    first, so the semantics are identical)."""
    ctr = [0]
    for f in nc.m.functions:
        for bb in f.blocks:
            new_insts = []
            for inst in bb.instructions:
                cap = 1
                w = inst.sync_info.on_wait if inst.sync_info else None
                if w and len(w) > cap:
                    waits = list(w)
                    extra, keep = waits[:-cap], waits[-cap:]
                    for i in range(0, len(extra), cap):
                        ctr[0] += 1
                        nop = mybir.InstNoOp(
                            name=f"WSPLIT-{ctr[0]}",
                            engine=inst.engine,
                            sync_info=mybir.SyncInfo(
                                on_wait=extra[i:i + cap], on_update=[]
                            ),
                        )
                        new_insts.append(nop)
                    inst.sync_info.on_wait = keep
                new_insts.append(inst)
            bb.instructions = new_insts
    return nc


_BUILT = None


def _get_built():
    global _BUILT
    if _BUILT is None:
        _BUILT = _split_excess_waits(build_bass())
    return _BUILT


def make_in_maps(x, pred_v_logits, pred_n_logits, w_proj_v, w_proj_n,
                 conv_w, conv_b, score_w, score_b):
    x = np.ascontiguousarray(np.asarray(x, np.float32).reshape(B_FULL, C, T * HW))
    pvf = np.asarray(pred_v_logits, np.float32)
    pnf = np.asarray(pred_n_logits, np.float32)
    wpvf = np.ascontiguousarray(np.asarray(w_proj_v, np.float32))
    wpnf = np.ascontiguousarray(np.asarray(w_proj_n, np.float32))
    cwtf = np.ascontiguousarray(
        np.asarray(conv_w, np.float32).reshape(D, C, 9).transpose(1, 2, 0)
        .reshape(C, 9 * D)).astype(_BF_NP)
    cbtf = np.ascontiguousarray(np.asarray(conv_b, np.float32).reshape(MT, 128).T)
    swtf = np.ascontiguousarray(np.asarray(score_w, np.float32).reshape(MT, 128).T)
    sbf = np.asarray(score_b, np.float32).reshape(1, 1)
    in_maps = []
    for i in range(N_CORES):
        sl = slice(B8 * i, B8 * (i + 1))
        in_maps.append({
            "x8": np.ascontiguousarray(x[sl]).astype(_BF_NP),
            "pv": np.ascontiguousarray(pvf[sl]),
            "pn": np.ascontiguousarray(pnf[sl]),
            "wpv": wpvf, "wpn": wpnf, "cwT": cwtf,
            "cbt": cbtf, "swt": swtf, "sb11": sbf,
        })
    return in_maps


def kernel(**inputs) -> np.ndarray:
    nc = _get_built()
    in_maps = make_in_maps(**inputs)
    res = run_bass_kernel_spmd(nc, in_maps, list(range(N_CORES)))
    total = 0.0
    for i in range(N_CORES):
        total += float(np.asarray(res.results[i]["out"], np.float64).sum())
    return np.float32(total / float(B_FULL * T * HW))


# revision 41
# speedup vs baseline: 1.0031x; 1.0031x over previous
"""Trainium2 Bass kernel for nn_CAMLocalBiHead (CAM scoring + topk mask + conv + BCE).

Strategy: pure data parallel over batch (8 samples per NeuronCore x 8 cores).
Each core computes per-sample BCE partial sums; host sums and divides.

Per-core pipeline:
  - conv weights [512,2304] f32 -> PE transposes -> lhsT layout [C,9,D] bf16
  - x pre-cast to bf16 on host, DMA'd straight into the [C, b, T, 7, 7] layout
  - CAM: argmax via reduce_max+is_ge one-hot, one-hot matmul gathers w_proj row,
    per-sample row einsum on PE (both cams in one M=2 matmul), min-max
    normalize, top-392 threshold by bisection (count = fused is_ge+accum DVE
    pass), mask = (r>=lo)*r, y = max(v, n)
  - conv 3x3 (C=256->D=512): 18 accumulating matmuls per [128, 392] PSUM tile
    with pad-skip tap subranges (boundary taps omitted == zero padding exactly)
  - relu+bias fused in ACT PSUM->SBUF evacuation (bf16)
  - 1x1 score conv: 4 accumulating [128,1]x[128,392] matmuls, deferred one
    group so PE never waits on ACT; nt-major order so BCE chunks overlap conv
  - BCE: sum(softplus(x) - y*x), softplus = relu(x) + ln(1+exp(-|x|)) on ACT
    with fused per-partition accumulation; emitted per column chunk inline
"""

import numpy as np

import concourse.bass as bass
import concourse.mybir as mybir

_BF_NP = None  # set below once mybir is imported
from concourse.tile import TileContext
from concourse.tile_rust import add_dep_helper
from concourse.masks import make_identity
from concourse.bass_utils import run_bass_kernel_spmd

FP = mybir.dt.float32
BF = mybir.dt.bfloat16
AF = mybir.ActivationFunctionType
OP = mybir.AluOpType
AX = mybir.AxisListType
_BF_NP = mybir.dt.np(mybir.dt.bfloat16)

N_CORES = 8
B_FULL = 64
C = 256
T = 32
H = 7
W = 7
HW = H * W
D = 512
KV = 97
KN = 300
NTOK = 392
B8 = B_FULL // N_CORES
KT = C // 128          # 2 contraction tiles
MT = D // 128          # 4 output-channel tiles
TCH = 8                # t-slices per spatial chunk
BISECT_ITERS = 14


def build_bass(b8=B8, t=T, ntok=NTOK, bisect_iters=BISECT_ITERS, pad_skip=True):
    n = t * HW                    # spatial positions per sample
    ntc = t // TCH                # number of spatial chunks
    chunk = TCH * HW              # 392 positions per chunk

    nc = bass.Bass()
    x8 = nc.declare_dram_parameter("x8", [b8, C, n], BF, isOutput=False)
    pv = nc.declare_dram_parameter("pv", [b8, KV], FP, isOutput=False)
    pn = nc.declare_dram_parameter("pn", [b8, KN], FP, isOutput=False)
    wpv = nc.declare_dram_parameter("wpv", [KV, C], FP, isOutput=False)
    wpn = nc.declare_dram_parameter("wpn", [KN, C], FP, isOutput=False)
    cwT = nc.declare_dram_parameter("cwT", [C, 9 * D], BF, isOutput=False)
    cbt = nc.declare_dram_parameter("cbt", [128, MT], FP, isOutput=False)
    swt = nc.declare_dram_parameter("swt", [128, MT], FP, isOutput=False)
    sb11 = nc.declare_dram_parameter("sb11", [1, 1], FP, isOutput=False)
    out = nc.declare_dram_parameter("out", [b8, 1], FP, isOutput=True)

    # pad-skip ranges per kernel tap: output h in [oh0, oh1), input h = h+ky-1
    def ranges(k):
        o0 = max(0, 1 - k)
        o1 = min(7, 8 - k)
        return o0, o1

    with TileContext(nc) as tc:
        with tc.tile_pool(name="cp", bufs=1) as cp:
            # ---- persistent tiles ----
            idn8 = cp.tile([b8, b8], FP)
            make_identity(nc, idn8[:])

            wT = [cp.tile([128, 9, D], BF, name=f"wT{k}") for k in range(KT)]
            # pad_skip: unpadded layout, boundary taps use restricted ranges
            # (exactly equivalent to zero padding). else: 9x9 zero-padded.
            PW = H if pad_skip else H + 2
            xbf = [cp.tile([128, b8, t, PW, PW], BF, name=f"xbf{k}")
                   for k in range(KT)]
            xoff = 0 if pad_skip else 1
            cb_t = cp.tile([128, MT], FP)
            sw_t = cp.tile([128, MT], FP)
            sT = cp.tile([128, MT], BF)
            sb_t = cp.tile([1, 1], FP)
            pv_t = cp.tile([b8, KV], FP)
            pn_t = cp.tile([b8, KN], FP)
            wpv_t = cp.tile([KV, C], FP)
            wpn_t = cp.tile([128, 3, C], FP)
            ohv = cp.tile([b8, KV], FP)
            ohn = cp.tile([b8, KN], FP)
            ohvT = cp.tile([KV, b8], FP)
            ohnT = cp.tile([128, 3, b8], FP)
            wtop = [cp.tile([b8, C], FP, name=f"wtop{i}") for i in range(2)]
            wtTa = cp.tile([128, KT, b8, 2], BF)
            r2 = 2 * b8
            rows = cp.tile([r2, n], FP)   # v rows first b8 partitions, n rows next
            cam16 = cp.tile([r2, n], FP)
            camn_s = cp.tile([b8, n], FP)
            y_t = cp.tile([b8, n], FP)
            xlog = cp.tile([b8, n], FP)
            # small stats (16 rows = both cams)
            mn16 = cp.tile([r2, 1], FP)
            mx16 = cp.tile([r2, 1], FP)
            rcp16 = cp.tile([r2, 1], FP)
            lo16 = cp.tile([r2, 1], FP)
            hi16 = cp.tile([r2, 1], FP)
            mid16 = cp.tile([r2, 1], FP)
            cnt16 = cp.tile([r2, 1], FP)
            prd16 = cp.tile([r2, 1], mybir.dt.uint32)
            vmv = cp.tile([b8, 1], FP)
            vmn = cp.tile([b8, 1], FP)
            scr_r = cp.tile([b8, chunk], FP)
            scr_l = cp.tile([b8, chunk], FP)
            scr_x = cp.tile([b8, chunk], FP)
            acc4r = cp.tile([b8, ntc], FP)
            acc4l = cp.tile([b8, ntc], FP)
            acc4x = cp.tile([b8, ntc], FP)
            partial = cp.tile([b8, 1], FP)

            # ---- small input DMAs ----
            nc.sync.dma_start(out=cb_t[:], in_=cbt[:])
            nc.sync.dma_start(out=sw_t[:], in_=swt[:])
            nc.sync.dma_start(out=sb_t[:], in_=sb11[:])
            nc.sync.dma_start(out=pv_t[:], in_=pv[:])
            nc.sync.dma_start(out=pn_t[:], in_=pn[:])
            nc.sync.dma_start(out=wpv_t[:], in_=wpv[:])
            kn_cnt = [128, 128, KN - 256]
            for i in range(3):
                nc.sync.dma_start(
                    out=wpn_t[0:kn_cnt[i], i, :], in_=wpn[128 * i:128 * i + kn_cnt[i], :]
                )
            nc.vector.tensor_copy(sT[:], sw_t[:])
            if not pad_skip:
                for kt in range(KT):
                    nc.vector.memset(xbf[kt][:], 0.0)

            # ---- PE warm-up: dummy accumulating matmuls on a zeroed tile
            # warm the HAM clock gate (4/8 -> 8/8) while input DMAs are still
            # in flight; transposes and DMA waits don't count as PE-busy, so
            # without this the whole CAM phase runs at 1.2 GHz. A second
            # burst after CAM prep keeps PE busy across the dependency wait
            # so the MID window never re-throttles.
            wup_cm = tc.tile_pool(name="wup", bufs=1, space="PSUM")
            wup = wup_cm.__enter__()
            wsrc = cp.tile([128, 512], BF)
            nc.vector.memset(wsrc[:], 0.0)
            wps = wup.tile([128, 512], FP)

            def warm_burst(k):
                for i in range(k):
                    nc.tensor.matmul(
                        wps[:], wsrc[:, 0:128], wsrc[:],
                        start=(i == 0), stop=(i == k - 1),
                    )

            warm_burst(24)

            # ---- weights arrive pre-transposed (and bf16) from the host ----
            for kt in range(KT):
                nc.sync.dma_start(
                    out=wT[kt][:].rearrange("p k d -> p (k d)"),
                    in_=cwT[128 * kt:128 * (kt + 1), :],
                )

            # ---- phase pool: CAM prep ----
            with tc.tile_pool(name="pst", bufs=4, space="PSUM") as pst:
                # CAM prep: argmax one-hot -> w_top -> transposed lhsT
                nc.vector.tensor_reduce(vmv[:], pv_t[:], axis=AX.X, op=OP.max)
                nc.vector.tensor_scalar(ohv[:], pv_t[:], vmv[:], None, op0=OP.is_ge)
                nc.vector.tensor_reduce(vmn[:], pn_t[:], axis=AX.X, op=OP.max)
                nc.vector.tensor_scalar(ohn[:], pn_t[:], vmn[:], None, op0=OP.is_ge)

                psv = pst.tile([KV, b8], FP, tag="pst")
                nc.tensor.transpose(out=psv[:], in_=ohv[:], identity=idn8[:])
                nc.vector.tensor_copy(ohvT[:], psv[:])
                for i in range(3):
                    psn = pst.tile([128, b8], FP, tag="pst")
                    nc.tensor.transpose(
                        out=psn[0:kn_cnt[i], :], in_=ohn[:, 128 * i:128 * i + kn_cnt[i]],
                        identity=idn8[:],
                    )
                    nc.vector.tensor_copy(ohnT[0:kn_cnt[i], i, :], psn[0:kn_cnt[i], :])

                warm_burst(14)
                pw = pst.tile([b8, C], FP, tag="pst")
                nc.tensor.matmul(pw[:], ohvT[:], wpv_t[:], start=True, stop=True)
                nc.vector.tensor_copy(wtop[0][:], pw[:])
                pw2 = pst.tile([b8, C], FP, tag="pst")
                for i in range(3):
                    nc.tensor.matmul(
                        pw2[:], ohnT[0:kn_cnt[i], i, :], wpn_t[0:kn_cnt[i], i, :],
                        start=(i == 0), stop=(i == 2),
                    )
                nc.vector.tensor_copy(wtop[1][:], pw2[:])

                for cam in range(2):
                    for kt in range(KT):
                        pt = pst.tile([128, b8], FP, tag="pst")
                        nc.tensor.transpose(
                            out=pt[:], in_=wtop[cam][:, 128 * kt:128 * (kt + 1)],
                            identity=idn8[:],
                        )
                        nc.scalar.copy(out=wtTa[:, kt, :, cam], in_=pt[:])

            # final filler before the CAM row matmuls
            warm_burst(6)
            wup_cm.__exit__(None, None, None)

            # ---- main loop pools ----
            with (
                tc.tile_pool(name="rowp", bufs=2, space="PSUM") as rowp,
                tc.tile_pool(name="convp", bufs=4, space="PSUM") as convp,
                tc.tile_pool(name="scp", bufs=2, space="PSUM") as scp,
                tc.tile_pool(name="h1p", bufs=12) as h1p,
                tc.tile_pool(name="bncp", bufs=4) as bncp,
            ):
                pending = []
                last_dma = None
                last_pe = None
                last_act = None
                last_dve = None

                def bce_chunk(c):
                    # BCE over column chunk c of all samples: emitted inline
                    # so ACT/DVE process it while conv continues (strict FIFO)
                    nonlocal last_act, last_dve
                    sl = slice(chunk * c, chunk * (c + 1))
                    nc.scalar.activation(
                        out=scr_r[:], in_=xlog[:, sl], func=AF.Relu,
                        accum_out=acc4r[:, c:c + 1],
                    )
                    nc.scalar.activation(
                        out=scr_l[:], in_=xlog[:, sl], func=AF.Abs
                    )
                    nc.scalar.activation(
                        out=scr_l[:], in_=scr_l[:], func=AF.Exp, scale=-1.0,
                    )
                    last_act = nc.scalar.activation(
                        out=scr_l[:], in_=scr_l[:], func=AF.Ln,
                        bias=1.0, accum_out=acc4l[:, c:c + 1],
                    )
                    nc.vector.tensor_tensor(
                        scr_x[:], y_t[:, sl], xlog[:, sl], op=OP.mult
                    )
                    last_dve = nc.vector.tensor_reduce(
                        acc4x[:, c:c + 1], scr_x[:], axis=AX.X, op=OP.add
                    )

                def emit_score(grp):
                    nonlocal last_pe, last_act, last_dma
                    gb, gnt, h1s = grp
                    sp_ps = scp.tile([1, chunk], FP, tag="scps")
                    for mt in range(MT):
                        last_pe = nc.tensor.matmul(
                            sp_ps[:], sT[:, mt:mt + 1], h1s[mt][:],
                            start=(mt == 0), stop=(mt == MT - 1),
                        )
                    # compute-engine SBUF writes must start at partition
                    # 0/32/64/96, so evacuate to a partition-0 bounce tile and
                    # DMA-shift into xlog[gb]
                    xb = bncp.tile([1, chunk], FP, tag="xb", name="xb")
                    last_act = nc.scalar.activation(
                        out=xb[:], in_=sp_ps[:], func=AF.Identity,
                        bias=sb_t[0:1, 0:1],
                    )
                    last_dma = nc.sync.dma_start(
                        out=xlog[gb:gb + 1, chunk * gnt:chunk * (gnt + 1)],
                        in_=xb[:],
                    )
                    if gb == b8 - 1:
                        bce_chunk(gnt)

                for b in range(b8):
                    # x arrives pre-cast to bf16 from the host; DMA straight
                    # into the conv layout
                    for kt in range(KT):
                        if pad_skip:
                            last_dma = nc.sync.dma_start(
                                out=xbf[kt][:, b, :, :, :]
                                    .rearrange("p t h w -> p (t h w)"),
                                in_=x8[b, 128 * kt:128 * (kt + 1), :],
                            )
                        else:
                            for ti in range(t):
                                last_dma = nc.sync.dma_start(
                                    out=xbf[kt][:, b, ti, xoff:xoff + H,
                                                xoff:xoff + W],
                                    in_=x8[b, 128 * kt:128 * (kt + 1),
                                           HW * ti:HW * (ti + 1)]
                                        .rearrange("p (h w) -> p h w", w=W),
                                )

                    # CAM row einsum: both cams in one matmul (M=2)
                    rb = bncp.tile([2, n], FP, tag="rb")
                    for nt in range(ntc):
                        rp = rowp.tile([2, chunk], FP, tag="rowps")
                        for kt in range(KT):
                            nc.tensor.matmul(
                                rp[:], wtTa[:, kt, b, :],
                                xbf[kt][:, b, TCH * nt:TCH * (nt + 1),
                                        xoff:xoff + H, xoff:xoff + W],
                                start=(kt == 0), stop=(kt == KT - 1),
                            )
                        nc.scalar.copy(
                            out=rb[0:2, chunk * nt:chunk * (nt + 1)], in_=rp[:]
                        )
                    nc.sync.dma_start(out=rows[b:b + 1, :], in_=rb[0:1, :])
                    nc.sync.dma_start(
                        out=rows[b8 + b:b8 + b + 1, :], in_=rb[1:2, :]
                    )

                # ---- CAM stats: normalize + bisection threshold + mask ----
                nc.vector.tensor_reduce(mn16[:], rows[:], axis=AX.X, op=OP.min)
                nc.vector.tensor_reduce(mx16[:], rows[:], axis=AX.X, op=OP.max)
                nc.vector.tensor_tensor(rcp16[:], mx16[:], mn16[:], op=OP.subtract)
                nc.vector.reciprocal(rcp16[:], rcp16[:])
                nc.vector.tensor_scalar(
                    rows[:], rows[:], mn16[:], rcp16[:],
                    op0=OP.subtract, op1=OP.mult,
                )
                nc.vector.memset(lo16[:], 0.0)
                nc.vector.memset(hi16[:], 1.0)
                for _ in range(bisect_iters):
                    nc.vector.tensor_tensor(mid16[:], lo16[:], hi16[:], op=OP.add)
                    nc.vector.tensor_scalar_mul(mid16[:], mid16[:], 0.5)
                    nc.vector.tensor_scalar(
                        cam16[:], rows[:], mid16[:], None, op0=OP.is_ge,
                        op1=OP.add, accum_out=cnt16[:],
                    )
                    nc.vector.tensor_scalar(
                        prd16[:], cnt16[:], float(ntok), None, op0=OP.is_ge
                    )
                    nc.vector.copy_predicated(lo16[:], prd16[:], mid16[:])
                    nc.vector.tensor_scalar(
                        prd16[:], cnt16[:], float(ntok), None, op0=OP.is_lt
                    )
                    nc.vector.copy_predicated(hi16[:], prd16[:], mid16[:])
                # cam = (r >= lo) * r for both cams at once
                nc.vector.scalar_tensor_tensor(
                    out=cam16[:], in0=rows[:], scalar=lo16[:], in1=rows[:],
                    op0=OP.is_ge, op1=OP.mult,
                )
                # shift n-cam rows to partitions 0-7, then y = max(v, n)
                shift_dma = nc.sync.dma_start(out=camn_s[:], in_=cam16[b8:r2, :])
                nc.vector.tensor_tensor(y_t[:], cam16[0:b8, :], camn_s[:], op=OP.max)

                # ---- conv 3x3 + deferred 1x1 score, all samples ----
                # (emitted after the CAM chain so the bisection DVE work
                # overlaps conv matmuls instead of trailing the kernel)
                taps = [(1, 1)] + [(ky, kx) for ky in range(3)
                                   for kx in range(3) if (ky, kx) != (1, 1)]
                for nt in range(ntc):
                    for b in range(b8):
                        if pending:
                            emit_score(pending.pop())
                        h1s = []
                        for mt in range(MT):
                            cps = convp.tile([128, chunk], FP, tag="cvps")
                            cpv = cps.rearrange("p (t h w) -> p t h w", h=H, w=W)
                            ntaps = 9 * KT
                            i = 0
                            for ky, kx in taps:
                                if pad_skip:
                                    oh0, oh1 = ranges(ky)
                                    ow0, ow1 = ranges(kx)
                                else:
                                    oh0, oh1, ow0, ow1 = 0, H, 0, W
                                for kt in range(KT):
                                    nc.tensor.matmul(
                                        cpv[:, :, oh0:oh1, ow0:ow1],
                                        wT[kt][:, 3 * ky + kx,
                                               128 * mt:128 * (mt + 1)],
                                        xbf[kt][:, b, TCH * nt:TCH * (nt + 1),
                                                xoff + oh0 + ky - 1:xoff + oh1 + ky - 1,
                                                xoff + ow0 + kx - 1:xoff + ow1 + kx - 1],
                                        start=(i == 0), stop=(i == ntaps - 1),
                                    )
                                    i += 1
                            h1t = h1p.tile([128, chunk], BF, tag="h1")
                            nc.scalar.activation(
                                out=h1t[:], in_=cps[:], func=AF.Relu,
                                bias=cb_t[:, mt:mt + 1], scale=1.0,
                            )
                            h1s.append(h1t)
                        pending.append((b, nt, h1s))
                if pending:
                    emit_score(pending.pop())

                # ---- final reduction of per-chunk BCE accumulators ----
                nc.vector.tensor_tensor(acc4r[:], acc4r[:], acc4l[:], op=OP.add)
                nc.vector.tensor_tensor(acc4r[:], acc4r[:], acc4x[:],
                                        op=OP.subtract)
                last_dve = nc.vector.tensor_reduce(
                    partial[:], acc4r[:], axis=AX.X, op=OP.add
                )
                out_dma = nc.sync.dma_start(out=out[:], in_=partial[:])

                tail = [last_dma, shift_dma, last_pe, last_act,
                        last_dve, out_dma]

                # funnel every engine's final tick through single-wait SP nops
                # so the TileContext tail drain needs <=2 sem waits (walrus
                # rejects instructions with more)
                prev = None
                for dep in tail:
                    if dep is None:
                        continue
                    nop = nc.sync.nop()
                    add_dep_helper(nop.ins, dep.ins, True, "tail funnel")
                    if prev is not None:
                        add_dep_helper(nop.ins, prev.ins, False, "tail chain")
                    prev = nop
    return nc


def _split_excess_waits(nc):
    """Walrus codegen rejects instructions with more sem waits than their
    ISA ctrl struct can hold (1 for Matmult via the LDWEIGHTS struct, ~2
    elsewhere). Hoist excess waits onto same-engine NOPs inserted right
    before the overloaded instruction (engine blocks on the NOP's waits
    first, so the semantics are identical)."""
    ctr = [0]
    for f in nc.m.functions:
        for bb in f.blocks:
            new_insts = []
            for inst in bb.instructions:
                cap = 0 if isinstance(inst, mybir.InstMatmult) else 1
                w = inst.sync_info.on_wait if inst.sync_info else None
                if w and len(w) > cap:
                    waits = list(w)
                    extra = waits[:-cap] if cap else waits
                    keep = waits[-cap:] if cap else []
                    for i in range(0, len(extra), max(cap, 1)):
                        ctr[0] += 1
                        nop = mybir.InstNoOp(
                            name=f"WSPLIT-{ctr[0]}",
                            engine=inst.engine,
                            sync_info=mybir.SyncInfo(
                                on_wait=extra[i:i + max(cap, 1)], on_update=[]
                            ),
                        )
                        new_insts.append(nop)
                    inst.sync_info.on_wait = keep
                new_insts.append(inst)
            bb.instructions = new_insts
    return nc


_BUILT = None


def _get_built():
    global _BUILT
    if _BUILT is None:
        _BUILT = _split_excess_waits(build_bass())
    return _BUILT


def make_in_maps(x, pred_v_logits, pred_n_logits, w_proj_v, w_proj_n,
                 conv_w, conv_b, score_w, score_b):
    x = np.ascontiguousarray(np.asarray(x, np.float32).reshape(B_FULL, C, T * HW))
    pvf = np.asarray(pred_v_logits, np.float32)
    pnf = np.asarray(pred_n_logits, np.float32)
    wpvf = np.ascontiguousarray(np.asarray(w_proj_v, np.float32))
    wpnf = np.ascontiguousarray(np.asarray(w_proj_n, np.float32))
    cwtf = np.ascontiguousarray(
        np.asarray(conv_w, np.float32).reshape(D, C, 9).transpose(1, 2, 0)
        .reshape(C, 9 * D)).astype(_BF_NP)
    cbtf = np.ascontiguousarray(np.asarray(conv_b, np.float32).reshape(MT, 128).T)
    swtf = np.ascontiguousarray(np.asarray(score_w, np.float32).reshape(MT, 128).T)
    sbf = np.asarray(score_b, np.float32).reshape(1, 1)
    in_maps = []
    for i in range(N_CORES):
        sl = slice(B8 * i, B8 * (i + 1))
        in_maps.append({
            "x8": np.ascontiguousarray(x[sl]).astype(_BF_NP),
            "pv": np.ascontiguousarray(pvf[sl]),
            "pn": np.ascontiguousarray(pnf[sl]),
            "wpv": wpvf, "wpn": wpnf, "cwT": cwtf,
            "cbt": cbtf, "swt": swtf, "sb11": sbf,
        })
    return in_maps


def kernel(**inputs) -> np.ndarray:
    nc = _get_built()
    in_maps = make_in_maps(**inputs)
    res = run_bass_kernel_spmd(nc, in_maps, list(range(N_CORES)))
    total = 0.0
    for i in range(N_CORES):
        total += float(np.asarray(res.results[i]["out"], np.float64).sum())
    return np.float32(total / float(B_FULL * T * HW))


# revision 43
# speedup vs baseline: 1.0062x; 1.0031x over previous
"""Trainium2 Bass kernel for nn_CAMLocalBiHead (CAM scoring + topk mask + conv + BCE).

Strategy: pure data parallel over batch (8 samples per NeuronCore x 8 cores).
Each core computes per-sample BCE partial sums; host sums and divides.

Per-core pipeline:
  - conv weights [512,2304] f32 -> PE transposes -> lhsT layout [C,9,D] bf16
  - x pre-cast to bf16 on host, DMA'd straight into the [C, b, T, 7, 7] layout
  - CAM: argmax via reduce_max+is_ge one-hot, one-hot matmul gathers w_proj row,
    per-sample row einsum on PE (both cams in one M=2 matmul), min-max
    normalize, top-392 threshold by bisection (count = fused is_ge+accum DVE
    pass), mask = (r>=lo)*r, y = max(v, n)
  - conv 3x3 (C=256->D=512): 18 accumulating matmuls per [128, 392] PSUM tile
    with pad-skip tap subranges (boundary taps omitted == zero padding exactly)
  - relu+bias fused in ACT PSUM->SBUF evacuation (bf16)
  - 1x1 score conv: 4 accumulating [128,1]x[128,392] matmuls, deferred one
    group so PE never waits on ACT; nt-major order so BCE chunks overlap conv
  - BCE: sum(softplus(x) - y*x), softplus = relu(x) + ln(1+exp(-|x|)) on ACT
    with fused per-partition accumulation; emitted per column chunk inline
"""

import numpy as np

import concourse.bass as bass
import concourse.mybir as mybir

_BF_NP = None  # set below once mybir is imported
from concourse.tile import TileContext
from concourse.tile_rust import add_dep_helper
from concourse.masks import make_identity
from concourse.bass_utils import run_bass_kernel_spmd

FP = mybir.dt.float32
BF = mybir.dt.bfloat16
AF = mybir.ActivationFunctionType
OP = mybir.AluOpType
AX = mybir.AxisListType
_BF_NP = mybir.dt.np(mybir.dt.bfloat16)

N_CORES = 8
B_FULL = 64
C = 256
T = 32
H = 7
W = 7
HW = H * W
D = 512
KV = 97
KN = 300
NTOK = 392
B8 = B_FULL // N_CORES
KT = C // 128          # 2 contraction tiles
MT = D // 128          # 4 output-channel tiles
TCH = 8                # t-slices per spatial chunk
BISECT_ITERS = 14


def build_bass(b8=B8, t=T, ntok=NTOK, bisect_iters=BISECT_ITERS, pad_skip=True):
    n = t * HW                    # spatial positions per sample
    ntc = t // TCH                # number of spatial chunks
    chunk = TCH * HW              # 392 positions per chunk

    nc = bass.Bass()
    x8 = nc.declare_dram_parameter("x8", [b8, C, n], BF, isOutput=False)
    pv = nc.declare_dram_parameter("pv", [b8, KV], FP, isOutput=False)
    pn = nc.declare_dram_parameter("pn", [b8, KN], FP, isOutput=False)
    wpv = nc.declare_dram_parameter("wpv", [KV, C], FP, isOutput=False)
    wpn = nc.declare_dram_parameter("wpn", [KN, C], FP, isOutput=False)
    cwT = nc.declare_dram_parameter("cwT", [C, 9 * D], BF, isOutput=False)
    cbt = nc.declare_dram_parameter("cbt", [128, MT], FP, isOutput=False)
    swt = nc.declare_dram_parameter("swt", [128, MT], FP, isOutput=False)
    sb11 = nc.declare_dram_parameter("sb11", [1, 1], FP, isOutput=False)
    out = nc.declare_dram_parameter("out", [b8, 1], FP, isOutput=True)

    # pad-skip ranges per kernel tap: output h in [oh0, oh1), input h = h+ky-1
    def ranges(k):
        o0 = max(0, 1 - k)
        o1 = min(7, 8 - k)
        return o0, o1

    with TileContext(nc) as tc:
        with tc.tile_pool(name="cp", bufs=1) as cp:
            # ---- persistent tiles ----
            idn8 = cp.tile([b8, b8], FP)
            make_identity(nc, idn8[:])

            wT = [cp.tile([128, 9, D], BF, name=f"wT{k}") for k in range(KT)]
            # pad_skip: unpadded layout, boundary taps use restricted ranges
            # (exactly equivalent to zero padding). else: 9x9 zero-padded.
            PW = H if pad_skip else H + 2
            xbf = [cp.tile([128, b8, t, PW, PW], BF, name=f"xbf{k}")
                   for k in range(KT)]
            xoff = 0 if pad_skip else 1
            cb_t = cp.tile([128, MT], FP)
            sw_t = cp.tile([128, MT], FP)
            sT = cp.tile([128, MT], BF)
            sb_t = cp.tile([1, 1], FP)
            pv_t = cp.tile([b8, KV], FP)
            pn_t = cp.tile([b8, KN], FP)
            wpv_t = cp.tile([KV, C], FP)
            wpn_t = cp.tile([128, 3, C], FP)
            ohv = cp.tile([b8, KV], FP)
            ohn = cp.tile([b8, KN], FP)
            ohvT = cp.tile([KV, b8], FP)
            ohnT = cp.tile([128, 3, b8], FP)
            wtop = [cp.tile([b8, C], FP, name=f"wtop{i}") for i in range(2)]
            wtTa = cp.tile([128, KT, b8, 2], BF)
            r2 = 2 * b8
            rows = cp.tile([r2, n], FP)   # v rows first b8 partitions, n rows next
            cam16 = cp.tile([r2, n], FP)
            camn_s = cp.tile([b8, n], FP)
            y_t = cp.tile([b8, n], FP)
            xlog = cp.tile([b8, n], FP)
            # small stats (16 rows = both cams)
            mn16 = cp.tile([r2, 1], FP)
            mx16 = cp.tile([r2, 1], FP)
            rcp16 = cp.tile([r2, 1], FP)
            lo16 = cp.tile([r2, 1], FP)
            hi16 = cp.tile([r2, 1], FP)
            mid16 = cp.tile([r2, 1], FP)
            cnt16 = cp.tile([r2, 1], FP)
            prd16 = cp.tile([r2, 1], mybir.dt.uint32)
            vmv = cp.tile([b8, 1], FP)
            vmn = cp.tile([b8, 1], FP)
            scr_r = cp.tile([b8, chunk], FP)
            scr_l = cp.tile([b8, chunk], FP)
            scr_x = cp.tile([b8, chunk], FP)
            acc4r = cp.tile([b8, ntc], FP)
            acc4l = cp.tile([b8, ntc], FP)
            acc4x = cp.tile([b8, ntc], FP)
            partial = cp.tile([b8, 1], FP)

            # ---- small input DMAs ----
            nc.sync.dma_start(out=cb_t[:], in_=cbt[:])
            nc.sync.dma_start(out=sw_t[:], in_=swt[:])
            nc.sync.dma_start(out=sb_t[:], in_=sb11[:])
            nc.sync.dma_start(out=pv_t[:], in_=pv[:])
            nc.sync.dma_start(out=pn_t[:], in_=pn[:])
            nc.sync.dma_start(out=wpv_t[:], in_=wpv[:])
            kn_cnt = [128, 128, KN - 256]
            for i in range(3):
                nc.sync.dma_start(
                    out=wpn_t[0:kn_cnt[i], i, :], in_=wpn[128 * i:128 * i + kn_cnt[i], :]
                )
            nc.vector.tensor_copy(sT[:], sw_t[:])
            if not pad_skip:
                for kt in range(KT):
                    nc.vector.memset(xbf[kt][:], 0.0)

            # ---- PE warm-up: dummy accumulating matmuls on a zeroed tile
            # warm the HAM clock gate (4/8 -> 8/8) while input DMAs are still
            # in flight; transposes and DMA waits don't count as PE-busy, so
            # without this the whole CAM phase runs at 1.2 GHz. A second
            # burst after CAM prep keeps PE busy across the dependency wait
            # so the MID window never re-throttles.
            wup_cm = tc.tile_pool(name="wup", bufs=1, space="PSUM")
            wup = wup_cm.__enter__()
            wsrc = cp.tile([128, 512], BF)
            nc.vector.memset(wsrc[:], 0.0)
            wps = wup.tile([128, 512], FP)

            def warm_burst(k):
                for i in range(k):
                    nc.tensor.matmul(
                        wps[:], wsrc[:, 0:128], wsrc[:],
                        start=(i == 0), stop=(i == k - 1),
                    )

            warm_burst(24)

            # ---- weights arrive pre-transposed (and bf16) from the host ----
            for kt in range(KT):
                nc.sync.dma_start(
                    out=wT[kt][:].rearrange("p k d -> p (k d)"),
                    in_=cwT[128 * kt:128 * (kt + 1), :],
                )

            # ---- phase pool: CAM prep ----
            with tc.tile_pool(name="pst", bufs=4, space="PSUM") as pst:
                # CAM prep: argmax one-hot -> w_top -> transposed lhsT
                nc.vector.tensor_reduce(vmv[:], pv_t[:], axis=AX.X, op=OP.max)
                nc.vector.tensor_scalar(ohv[:], pv_t[:], vmv[:], None, op0=OP.is_ge)
                nc.vector.tensor_reduce(vmn[:], pn_t[:], axis=AX.X, op=OP.max)
                nc.vector.tensor_scalar(ohn[:], pn_t[:], vmn[:], None, op0=OP.is_ge)

                psv = pst.tile([KV, b8], FP, tag="pst")
                nc.tensor.transpose(out=psv[:], in_=ohv[:], identity=idn8[:])
                nc.vector.tensor_copy(ohvT[:], psv[:])
                for i in range(3):
                    psn = pst.tile([128, b8], FP, tag="pst")
                    nc.tensor.transpose(
                        out=psn[0:kn_cnt[i], :], in_=ohn[:, 128 * i:128 * i + kn_cnt[i]],
                        identity=idn8[:],
                    )
                    nc.vector.tensor_copy(ohnT[0:kn_cnt[i], i, :], psn[0:kn_cnt[i], :])

                warm_burst(14)
                pw = pst.tile([b8, C], FP, tag="pst")
                nc.tensor.matmul(pw[:], ohvT[:], wpv_t[:], start=True, stop=True)
                nc.vector.tensor_copy(wtop[0][:], pw[:])
                pw2 = pst.tile([b8, C], FP, tag="pst")
                for i in range(3):
                    nc.tensor.matmul(
                        pw2[:], ohnT[0:kn_cnt[i], i, :], wpn_t[0:kn_cnt[i], i, :],
                        start=(i == 0), stop=(i == 2),
                    )
                nc.vector.tensor_copy(wtop[1][:], pw2[:])

                for cam in range(2):
                    for kt in range(KT):
                        pt = pst.tile([128, b8], FP, tag="pst")
                        nc.tensor.transpose(
                            out=pt[:], in_=wtop[cam][:, 128 * kt:128 * (kt + 1)],
                            identity=idn8[:],
                        )
                        nc.scalar.copy(out=wtTa[:, kt, :, cam], in_=pt[:])

            # final filler before the CAM row matmuls
            warm_burst(6)
            wup_cm.__exit__(None, None, None)

            # ---- main loop pools ----
            with (
                tc.tile_pool(name="rowp", bufs=2, space="PSUM") as rowp,
                tc.tile_pool(name="convp", bufs=4, space="PSUM") as convp,
                tc.tile_pool(name="scp", bufs=2, space="PSUM") as scp,
                tc.tile_pool(name="h1p", bufs=12) as h1p,
                tc.tile_pool(name="bncp", bufs=4) as bncp,
            ):
                pending = []
                last_dma = None
                last_pe = None
                last_act = None
                last_dve = None

                def bce_chunk(c):
                    # BCE over column chunk c of all samples: emitted inline
                    # so ACT/DVE process it while conv continues (strict FIFO)
                    nonlocal last_act, last_dve
                    sl = slice(chunk * c, chunk * (c + 1))
                    nc.scalar.activation(
                        out=scr_r[:], in_=xlog[:, sl], func=AF.Relu,
                        accum_out=acc4r[:, c:c + 1],
                    )
                    nc.scalar.activation(
                        out=scr_l[:], in_=xlog[:, sl], func=AF.Abs
                    )
                    nc.scalar.activation(
                        out=scr_l[:], in_=scr_l[:], func=AF.Exp, scale=-1.0,
                    )
                    last_act = nc.scalar.activation(
                        out=scr_l[:], in_=scr_l[:], func=AF.Ln,
                        bias=1.0, accum_out=acc4l[:, c:c + 1],
                    )
                    nc.vector.tensor_tensor(
                        scr_x[:], y_t[:, sl], xlog[:, sl], op=OP.mult
                    )
                    last_dve = nc.vector.tensor_reduce(
                        acc4x[:, c:c + 1], scr_x[:], axis=AX.X, op=OP.add
                    )

                def emit_score(grp):
                    nonlocal last_pe, last_act, last_dma
                    gb, gnt, h1s = grp
                    sp_ps = scp.tile([1, chunk], FP, tag="scps")
                    for mt in range(MT):
                        last_pe = nc.tensor.matmul(
                            sp_ps[:], sT[:, mt:mt + 1], h1s[mt][:],
                            start=(mt == 0), stop=(mt == MT - 1),
                        )
                    # compute-engine SBUF writes must start at partition
                    # 0/32/64/96, so evacuate to a partition-0 bounce tile and
                    # DMA-shift into xlog[gb]
                    xb = bncp.tile([1, chunk], FP, tag="xb", name="xb")
                    last_act = nc.scalar.activation(
                        out=xb[:], in_=sp_ps[:], func=AF.Identity,
                        bias=sb_t[0:1, 0:1],
                    )
                    last_dma = nc.sync.dma_start(
                        out=xlog[gb:gb + 1, chunk * gnt:chunk * (gnt + 1)],
                        in_=xb[:],
                    )
                    if gb == b8 - 1:
                        bce_chunk(gnt)

                for b in range(b8):
                    # x arrives pre-cast to bf16 from the host; DMA straight
                    # into the conv layout
                    for kt in range(KT):
                        if pad_skip:
                            last_dma = nc.sync.dma_start(
                                out=xbf[kt][:, b, :, :, :]
                                    .rearrange("p t h w -> p (t h w)"),
                                in_=x8[b, 128 * kt:128 * (kt + 1), :],
                            )
                        else:
                            for ti in range(t):
                                last_dma = nc.sync.dma_start(
                                    out=xbf[kt][:, b, ti, xoff:xoff + H,
                                                xoff:xoff + W],
                                    in_=x8[b, 128 * kt:128 * (kt + 1),
                                           HW * ti:HW * (ti + 1)]
                                        .rearrange("p (h w) -> p h w", w=W),
                                )

                    # CAM row einsum: both cams in one matmul (M=2)
                    rb = bncp.tile([2, n], FP, tag="rb")
                    for nt in range(ntc):
                        rp = rowp.tile([2, chunk], FP, tag="rowps")
                        for kt in range(KT):
                            nc.tensor.matmul(
                                rp[:], wtTa[:, kt, b, :],
                                xbf[kt][:, b, TCH * nt:TCH * (nt + 1),
                                        xoff:xoff + H, xoff:xoff + W],
                                start=(kt == 0), stop=(kt == KT - 1),
                            )
                        nc.scalar.copy(
                            out=rb[0:2, chunk * nt:chunk * (nt + 1)], in_=rp[:]
                        )
                    nc.sync.dma_start(out=rows[b:b + 1, :], in_=rb[0:1, :])
                    nc.sync.dma_start(
                        out=rows[b8 + b:b8 + b + 1, :], in_=rb[1:2, :]
                    )

                # ---- CAM stats: normalize + bisection threshold + mask ----
                nc.vector.tensor_reduce(mn16[:], rows[:], axis=AX.X, op=OP.min)
                nc.vector.tensor_reduce(mx16[:], rows[:], axis=AX.X, op=OP.max)
                nc.vector.tensor_tensor(rcp16[:], mx16[:], mn16[:], op=OP.subtract)
                nc.vector.reciprocal(rcp16[:], rcp16[:])
                nc.vector.tensor_scalar(
                    rows[:], rows[:], mn16[:], rcp16[:],
                    op0=OP.subtract, op1=OP.mult,
                )
                nc.vector.memset(lo16[:], 0.0)
                nc.vector.memset(hi16[:], 1.0)
                for _ in range(bisect_iters):
                    nc.vector.tensor_tensor(mid16[:], lo16[:], hi16[:], op=OP.add)
                    nc.vector.tensor_scalar_mul(mid16[:], mid16[:], 0.5)
                    nc.vector.tensor_scalar(
                        cam16[:], rows[:], mid16[:], None, op0=OP.is_ge,
                        op1=OP.add, accum_out=cnt16[:],
                    )
                    nc.vector.tensor_scalar(
                        prd16[:], cnt16[:], float(ntok), None, op0=OP.is_ge
                    )
                    nc.vector.copy_predicated(lo16[:], prd16[:], mid16[:])
                    nc.vector.tensor_scalar(
                        prd16[:], cnt16[:], float(ntok), None, op0=OP.is_lt
                    )
                    nc.vector.copy_predicated(hi16[:], prd16[:], mid16[:])
                # cam = (r >= lo) * r for both cams at once
                nc.vector.scalar_tensor_tensor(
                    out=cam16[:], in0=rows[:], scalar=lo16[:], in1=rows[:],
                    op0=OP.is_ge, op1=OP.mult,
                )
                # shift n-cam rows to partitions 0-7, then y = max(v, n)
                shift_dma = nc.sync.dma_start(out=camn_s[:], in_=cam16[b8:r2, :])
                nc.vector.tensor_tensor(y_t[:], cam16[0:b8, :], camn_s[:], op=OP.max)

                # ---- conv 3x3 + deferred 1x1 score, all samples ----
                # (emitted after the CAM chain so the bisection DVE work
                # overlaps conv matmuls instead of trailing the kernel)
                taps = [(1, 1)] + [(ky, kx) for ky in range(3)
                                   for kx in range(3) if (ky, kx) != (1, 1)]
                for nt in range(ntc):
                    for b in range(b8):
                        if pending:
                            emit_score(pending.pop())
                        h1s = []
                        for mt in range(MT):
                            cps = convp.tile([128, chunk], FP, tag="cvps")
                            cpv = cps.rearrange("p (t h w) -> p t h w", h=H, w=W)
                            ntaps = 9 * KT
                            i = 0
                            for ky, kx in taps:
                                if pad_skip:
                                    oh0, oh1 = ranges(ky)
                                    ow0, ow1 = ranges(kx)
                                else:
                                    oh0, oh1, ow0, ow1 = 0, H, 0, W
                                for kt in range(KT):
                                    nc.tensor.matmul(
                                        cpv[:, :, oh0:oh1, ow0:ow1],
                                        wT[kt][:, 3 * ky + kx,
                                               128 * mt:128 * (mt + 1)],
                                        xbf[kt][:, b, TCH * nt:TCH * (nt + 1),
                                                xoff + oh0 + ky - 1:xoff + oh1 + ky - 1,
                                                xoff + ow0 + kx - 1:xoff + ow1 + kx - 1],
                                        start=(i == 0), stop=(i == ntaps - 1),
                                    )
                                    i += 1
                            h1t = h1p.tile([128, chunk], BF, tag="h1")
                            nc.scalar.activation(
                                out=h1t[:], in_=cps[:], func=AF.Relu,
                                bias=cb_t[:, mt:mt + 1], scale=1.0,
                            )
                            h1s.append(h1t)
                        pending.append((b, nt, h1s))
                if pending:
                    emit_score(pending.pop())

                # ---- final reduction of per-chunk BCE accumulators ----
                nc.vector.tensor_tensor(acc4r[:], acc4r[:], acc4l[:], op=OP.add)
                nc.vector.tensor_tensor(acc4r[:], acc4r[:], acc4x[:],
                                        op=OP.subtract)
                last_dve = nc.vector.tensor_reduce(
                    partial[:], acc4r[:], axis=AX.X, op=OP.add
                )
                out_dma = nc.sync.dma_start(out=out[:], in_=partial[:])

                tail = [last_dma, shift_dma, last_pe, last_act,
                        last_dve, out_dma]

                # funnel every engine's final tick through single-wait SP nops
                # so the TileContext tail drain needs <=2 sem waits (walrus
                # rejects instructions with more)
                prev = None
                for dep in tail:
                    if dep is None:
                        continue
                    nop = nc.sync.nop()
                    add_dep_helper(nop.ins, dep.ins, True, "tail funnel")
                    if prev is not None:
                        add_dep_helper(nop.ins, prev.ins, False, "tail chain")
                    prev = nop
    return nc


def _split_excess_waits(nc):
    """Walrus codegen rejects instructions with more sem waits than their
    ISA ctrl struct can hold (1 for Matmult via the LDWEIGHTS struct, ~2
    elsewhere). Hoist excess waits onto same-engine NOPs inserted right
    before the overloaded instruction (engine blocks on the NOP's waits
    first, so the semantics are identical)."""
    ctr = [0]
    for f in nc.m.functions:
        for bb in f.blocks:
            new_insts = []
            for inst in bb.instructions:
                cap = 0 if isinstance(inst, mybir.InstMatmult) else 1
                w = inst.sync_info.on_wait if inst.sync_info else None
                if w and len(w) > cap:
                    waits = list(w)
                    extra = waits[:-cap] if cap else waits
                    keep = waits[-cap:] if cap else []
                    for i in range(0, len(extra), max(cap, 1)):
                        ctr[0] += 1
                        nop = mybir.InstNoOp(
                            name=f"WSPLIT-{ctr[0]}",
                            engine=inst.engine,
                            sync_info=mybir.SyncInfo(
                                on_wait=extra[i:i + max(cap, 1)], on_update=[]
                            ),
                        )
                        new_insts.append(nop)
                    inst.sync_info.on_wait = keep
                new_insts.append(inst)
            bb.instructions = new_insts
    return nc


_BUILT = None


def _get_built():
    global _BUILT
    if _BUILT is None:
        _BUILT = _split_excess_waits(build_bass())
    return _BUILT


def make_in_maps(x, pred_v_logits, pred_n_logits, w_proj_v, w_proj_n,
                 conv_w, conv_b, score_w, score_b):
    x = np.ascontiguousarray(np.asarray(x, np.float32).reshape(B_FULL, C, T * HW))
    pvf = np.asarray(pred_v_logits, np.float32)
    pnf = np.asarray(pred_n_logits, np.float32)
    wpvf = np.ascontiguousarray(np.asarray(w_proj_v, np.float32))
    wpnf = np.ascontiguousarray(np.asarray(w_proj_n, np.float32))
    cwtf = np.ascontiguousarray(
        np.asarray(conv_w, np.float32).reshape(D, C, 9).transpose(1, 2, 0)
        .reshape(C, 9 * D)).astype(_BF_NP)
    cbtf = np.ascontiguousarray(np.asarray(conv_b, np.float32).reshape(MT, 128).T)
    swtf = np.ascontiguousarray(np.asarray(score_w, np.float32).reshape(MT, 128).T)
    sbf = np.asarray(score_b, np.float32).reshape(1, 1)
    in_maps = []
    for i in range(N_CORES):
        sl = slice(B8 * i, B8 * (i + 1))
        in_maps.append({
            "x8": np.ascontiguousarray(x[sl]).astype(_BF_NP),
            "pv": np.ascontiguousarray(pvf[sl]),
            "pn": np.ascontiguousarray(pnf[sl]),
            "wpv": wpvf, "wpn": wpnf, "cwT": cwtf,
            "cbt": cbtf, "swt": swtf, "sb11": sbf,
        })
    return in_maps


def kernel(**inputs) -> np.ndarray:
    nc = _get_built()
    in_maps = make_in_maps(**inputs)
    res = run_bass_kernel_spmd(nc, in_maps, list(range(N_CORES)))
    total = 0.0
    for i in range(N_CORES):
        total += float(np.asarray(res.results[i]["out"], np.float64).sum())
    return np.float32(total / float(B_FULL * T * HW))
